# revision 1
# baseline (speedup 1.0000x reference)
"""Trainium2 Bass kernel for nn_DistillationStudentModel (per-view adapter MLP).

Math (per sample b with view v = idx[b]):
    xn  = LayerNorm(x; gamma[v], beta[v])
    h   = gelu(xn @ W1[v] + b1[v])          (erf gelu)
    out = x + h @ W2[v] + b2[v]

Strategy: shard the MLP hidden dim H=8192 across the 8 cores (HS=1024 each).
Every core processes ALL tokens with its H-slice of W1/W2 for all 3 views and
emits a partial MLP output; the host sums the 8 partials and adds the
residual x and b2.

Device-side layout is D-major ("transposed activations"): x is passed as
xT [D, T] so the mm1 contraction dim D sits on SBUF partitions, mm1 emits
hT [HS, T] with the mm2 contraction dim HS already on partitions, and mm2
emits poutT [D, T].

The tiny per-token LayerNorm stats (mu, rstd — 0.1% of the FLOPs) are
precomputed on the host and DMA-broadcast across partitions; the device
applies the normalization, runs both matmuls in bf16 (fp32 PSUM
accumulation), and the erf-GELU on the scalar engine. gamma is folded into
W1 and beta into b1 on the host (b1' = b1 + beta @ W1).

Samples are sorted by view on the host so each view's weight slice is loaded
into SBUF once; the token-tile plan (which view, tile length 512 or 256) is
baked into the compiled kernel from the actual indices.
"""

import numpy as np
import ml_dtypes

import concourse.bass as bass
import concourse.tile as tile
from concourse import bacc, mybir
from concourse.bass_utils import run_bass_kernel_spmd

B, P, D, H, V = 32, 256, 2048, 8192, 3
NCORES = 8
HS = H // NCORES          # per-core hidden slice
T = B * P                 # total tokens
KD = D // 128             # mm1 contraction subtiles
KH = HS // 128            # mm2 contraction subtiles
MH = HS // 128            # mm1 output row tiles
MD = D // 128             # mm2 output row tiles
NT = 512                  # tokens per tile (2 samples)
LN_EPS = 1e-5

f32 = mybir.dt.float32
bf16 = mybir.dt.bfloat16

# debugging/profiling hooks (unused by the grading path)
LAST_NC = None
LAST_RESULT = None


def _tile_plan(idx_sorted):
    """[(view, tok_offset, n_tokens)] with n_tokens in {512, 256}, aligned to
    sorted sample groups so every tile is single-view."""
    counts = np.bincount(idx_sorted, minlength=V)
    plan = []
    off = 0
    for v in range(V):
        n = int(counts[v])
        for _ in range(n // 2):
            plan.append((v, off, 2 * P))
            off += 2 * P
        if n % 2:
            plan.append((v, off, P))
            off += P
    assert off == T
    return plan


def _bcast_ap(handle_ap, toff, nt):
    """[128, nt] partition-stride-0 view of a 1-D DRAM tensor slice."""
    sl = handle_ap[toff:toff + nt]
    return bass.AP(tensor=sl.tensor, offset=sl.offset,
                   ap=[[0, 128]] + [list(p) for p in sl.ap])


def build(plan, repeats=1):
    nc = bacc.Bacc("TRN2", debug=False, num_devices=NCORES)
    x = nc.dram_tensor("xT", [D, T], f32, kind="ExternalInput")
    mu = nc.dram_tensor("mu", [T], f32, kind="ExternalInput")
    rstd = nc.dram_tensor("rstd", [T], f32, kind="ExternalInput")
    w1 = nc.dram_tensor("w1", [V, D, HS], bf16, kind="ExternalInput")
    b1 = nc.dram_tensor("b1", [V, HS], f32, kind="ExternalInput")
    w2 = nc.dram_tensor("w2", [V, HS, D], bf16, kind="ExternalInput")
    out = nc.dram_tensor("poutT", [D, T], f32, kind="ExternalOutput")

    x3 = x[:].rearrange("(k p) t -> p k t", p=128)
    w14 = w1[:].rearrange("v (k p) h -> p v k h", p=128)
    w24 = w2[:].rearrange("v (k p) d -> p v k d", p=128)
    b13 = b1[:].rearrange("v (m p) -> p v m", p=128)
    out3 = out[:].rearrange("(m p) t -> p m t", p=128)
    mu1 = mu[:]
    rstd1 = rstd[:]

    views_in_plan = []
    for v, _, _ in plan:
        if v not in views_in_plan:
            views_in_plan.append(v)

    with tile.TileContext(nc) as tc:
        with (
            tc.tile_pool(name="consts", bufs=1) as consts,
            tc.tile_pool(name="w1pool", bufs=18) as w1pool,
            tc.tile_pool(name="w2pool", bufs=10) as w2pool,
            tc.tile_pool(name="xpool", bufs=8) as xpool,
            tc.tile_pool(name="zpool", bufs=2) as zpool,
            tc.tile_pool(name="hpool", bufs=3) as hpool,
            tc.tile_pool(name="bcpool", bufs=2) as bcpool,
            tc.tile_pool(name="tpool", bufs=4) as tpool,
            tc.tile_pool(name="opool", bufs=4) as opool,
            tc.tile_pool(name="pmm", bufs=8, space="PSUM") as pmm,
        ):
            b1t = consts.tile([128, V, MH], f32)
            nc.sync.dma_start(b1t[:], b13)

            for _rep in range(repeats):
              for v in views_in_plan:
                w1k = [w1pool.tile([128, HS], bf16, tag="w1k", name=f"w1k_{_rep}_{v}_{k}")
                       for k in range(KD)]
                w2k = [w2pool.tile([128, D], bf16, tag="w2k", name=f"w2k_{_rep}_{v}_{k}")
                       for k in range(KH)]
                first_tile = True

                for (pv, toff, nt) in plan:
                    if pv != v:
                        continue
                    ts_ = slice(toff, toff + nt)

                    mean_bc = bcpool.tile([128, NT], f32, tag="mean_bc")
                    rstd_bc = bcpool.tile([128, NT], f32, tag="rstd_bc")
                    nc.sync.dma_start(mean_bc[:, :nt], _bcast_ap(mu1, toff, nt))
                    nc.sync.dma_start(rstd_bc[:, :nt], _bcast_ap(rstd1, toff, nt))

                    zt = zpool.tile([128, KD, NT], bf16, tag="zt")
                    for k in range(KD):
                        xt = xpool.tile([128, NT], f32, tag="xt")
                        nc.sync.dma_start(xt[:, :nt], x3[:, k, ts_])
                        tmp = tpool.tile([128, NT], f32, tag="tmp")
                        nc.vector.tensor_sub(tmp[:, :nt], xt[:, :nt],
                                             mean_bc[:, :nt])
                        nc.vector.tensor_mul(zt[:, k, :nt], tmp[:, :nt],
                                             rstd_bc[:, :nt])
                        if first_tile:
                            # interleave this view's W1 loads with the first
                            # tile's x/z stage so mm1 isn't starved at startup
                            nc.sync.dma_start(w1k[k][:], w14[:, v, k, :])
                    if first_tile:
                        # W2 is first needed by mm2, one mm1-phase later
                        for k in range(KH):
                            nc.sync.dma_start(w2k[k][:], w24[:, v, k, :])
                        first_tile = False

                    ht = hpool.tile([128, KH, NT], bf16, tag="ht")
                    for m in range(MH):
                        ph = pmm.tile([128, NT], f32, tag="mm")
                        for k in range(KD):
                            nc.tensor.matmul(ph[:, :nt],
                                             w1k[k][:, bass.ts(m, 128)],
                                             zt[:, k, :nt],
                                             start=(k == 0), stop=(k == KD - 1))
                        nc.scalar.activation(ht[:, m, :nt], ph[:, :nt],
                                             mybir.ActivationFunctionType.Gelu,
                                             bias=b1t[:, v, m:m + 1], scale=1.0)

                    for dsub in range(MD):
                        po = pmm.tile([128, NT], f32, tag="mm")
                        for k in range(KH):
                            nc.tensor.matmul(po[:, :nt],
                                             w2k[k][:, bass.ts(dsub, 128)],
                                             ht[:, k, :nt],
                                             start=(k == 0), stop=(k == KH - 1))
                        ot = opool.tile([128, NT], f32, tag="ot")
                        nc.vector.tensor_copy(ot[:, :nt], po[:, :nt])
                        nc.sync.dma_start(out3[:, dsub, ts_], ot[:, :nt])
    nc.finalize()
    return nc


def kernel(**inputs):
    x = np.asarray(inputs["vision_features"], dtype=np.float32)    # [B, P, D]
    idx = np.asarray(inputs["student_view_indices"]).astype(np.int64)  # [B]
    gamma = np.asarray(inputs["gamma"], dtype=np.float32)          # [V, D]
    beta = np.asarray(inputs["beta"], dtype=np.float32)            # [V, D]
    W1 = np.asarray(inputs["W1"], dtype=np.float32)                # [V, D, H]
    b1 = np.asarray(inputs["b1"], dtype=np.float32)                # [V, H]
    W2 = np.asarray(inputs["W2"], dtype=np.float32)                # [V, H, D]
    b2 = np.asarray(inputs["b2"], dtype=np.float32)                # [V, D]

    order = np.argsort(idx, kind="stable")
    idx_sorted = idx[order]
    plan = _tile_plan(idx_sorted)

    # host-side folds: gamma into W1 rows, beta into b1
    W1f = gamma[:, :, None] * W1                                   # [V, D, H]
    b1f = b1 + np.einsum("vd,vdh->vh", beta, W1)                   # [V, H]

    xs = x[order].reshape(T, D)                                    # sorted tokens
    xT = np.ascontiguousarray(xs.T)                                # [D, T]

    # per-token LayerNorm stats (fp64 accumulate)
    mu_t = xs.mean(axis=1, dtype=np.float64)
    ex2 = np.einsum("td,td->t", xs.astype(np.float64), xs.astype(np.float64)) / D
    var = ex2 - mu_t * mu_t
    rstd_t = (1.0 / np.sqrt(var + LN_EPS)).astype(np.float32)
    mu_t = mu_t.astype(np.float32)

    W1bf = W1f.astype(ml_dtypes.bfloat16)
    W2bf = W2.astype(ml_dtypes.bfloat16)

    in_maps = []
    for c in range(NCORES):
        hsl = slice(c * HS, (c + 1) * HS)
        in_maps.append({
            "xT": xT,
            "mu": mu_t,
            "rstd": rstd_t,
            "w1": np.ascontiguousarray(W1bf[:, :, hsl]),
            "b1": np.ascontiguousarray(b1f[:, hsl]),
            "w2": np.ascontiguousarray(W2bf[:, hsl, :]),
        })

    nc = build(plan)
    res = run_bass_kernel_spmd(nc, in_maps, core_ids=list(range(NCORES)))
    global LAST_NC, LAST_RESULT
    LAST_NC = nc
    LAST_RESULT = res

    pout = res.results[0]["poutT"].astype(np.float32).copy()
    for c in range(1, NCORES):
        pout += res.results[c]["poutT"]

    out_sorted = xs + pout.T                                       # [T, D]
    out_sorted += b2[np.repeat(idx_sorted, P)]
    out = np.empty((B, P, D), dtype=np.float32)
    out[order] = out_sorted.reshape(B, P, D)
    return out



# revision 2
# speedup vs baseline: 1.4783x; 1.4783x over previous
"""Trainium2 Bass kernel for nn_DistillationStudentModel (per-view adapter MLP).

Math (per sample b with view v = idx[b]):
    xn  = LayerNorm(x; gamma[v], beta[v])
    h   = gelu(xn @ W1[v] + b1[v])          (erf gelu)
    out = x + h @ W2[v] + b2[v]

Strategy: shard the MLP hidden dim H=8192 across the 8 cores (HS=1024 each);
every core processes all tokens against its H-slice and emits a partial mm2
output that the host reduces (plus residual x and biases).

The matmuls run in fp8-e4m3 with perf_mode=DoubleRow (K=256 per matmul, 2
PE MACs/cell/cycle).  Plain fp8 is far outside the 2e-2 tolerance, so each
matmul is augmented with first-order correction matmuls:

    z @ W  ~=  Q(z)@Q(W) + Q(z - Q(z))@Q(W) + Q(z)@Q(W*64 - Q(W*64))/64

mm1 keeps both corrections (activation delta dz and weight delta dW1) ->
numerically exact to ~1e-3.  mm2 keeps only the weight correction dW2; the
h-quantization error is shrunk by quantizing (h - 0.3) (the gelu output is
skewed positive, so a constant shift reduces relative quantization error)
and the dropped shift term 0.3*colsum(W2) is added back exactly on the host.
Measured end-to-end rel err ~1.7e-2 (budget 2e-2) vs 874us -> ~550us of PE
time (mm1: 1.5 cyc per K=256 column, mm2: 1.0 vs bf16's 2.0).

Weights are pre-scaled by 64 (power of two, exact) so fp8's normal range
covers them; the 1/64 is folded into the gelu's input scale and the host
reduction.  LayerNorm stats and quantization run on the host; z arrives
pre-packed as fp8 pairs.  Samples are sorted by view so each view's weights
are loaded once; the token-tile plan is baked into the compiled program.
The per-tile schedule is software-pipelined one tile deep (mm1 of tile i+1
is emitted before mm2 of tile i) so the PE never waits for the
gelu->quantize chain.
"""

import numpy as np
import ml_dtypes

import concourse.bass as bass
import concourse.tile as tile
from concourse import bacc, mybir
from concourse.bass_utils import run_bass_kernel_spmd

B, P, D, H, V = 32, 256, 2048, 8192, 3
NCORES = 8
HS = H // NCORES          # per-core hidden slice
T = B * P                 # total tokens
KC1 = D // 256            # mm1 DoubleRow contraction chunks
MH = HS // 128            # mm1 output row tiles
KC2 = HS // 256           # mm2 DoubleRow contraction chunks
MD = D // 128             # mm2 output row tiles
NT = 512                  # tokens per tile (2 samples)
LN_EPS = 1e-5
SW = 64.0                 # weight prescale (power of two)
C_SHIFT = 0.30            # h quantization shift

f32 = mybir.dt.float32
bf16 = mybir.dt.bfloat16
f16 = mybir.dt.float16
f8 = mybir.dt.float8e4
E4 = ml_dtypes.float8_e4m3
DR = mybir.MatmulPerfMode.DoubleRow

# debugging/profiling hooks (unused by the grading path)
LAST_NC = None
LAST_RESULT = None


def _tile_plan(idx_sorted):
    """[(view, tok_offset, n_tokens)] with n_tokens in {512, 256}, aligned to
    sorted sample groups so every tile is single-view."""
    counts = np.bincount(idx_sorted, minlength=V)
    plan = []
    off = 0
    for v in range(V):
        n = int(counts[v])
        for _ in range(n // 2):
            plan.append((v, off, 2 * P))
            off += 2 * P
        if n % 2:
            plan.append((v, off, P))
            off += P
    assert off == T
    return plan


def build(plan):
    nc = bacc.Bacc("TRN2", debug=False, num_devices=NCORES)
    zq_d = nc.dram_tensor("zq", [128, KC1, 2, T], f8, kind="ExternalInput")
    dz_d = nc.dram_tensor("dz", [128, KC1, 2, T], f8, kind="ExternalInput")
    w1_d = nc.dram_tensor("w1q", [V, 128, KC1, 2, HS], f8, kind="ExternalInput")
    dw1_d = nc.dram_tensor("dw1", [V, 128, KC1, 2, HS], f8, kind="ExternalInput")
    w2_d = nc.dram_tensor("w2q", [V, 128, KC2, 2, D], f8, kind="ExternalInput")
    dw2_d = nc.dram_tensor("dw2", [V, 128, KC2, 2, D], f8, kind="ExternalInput")
    b1_d = nc.dram_tensor("b1", [128, V, MH], f32, kind="ExternalInput")
    out_d = nc.dram_tensor("pout", [128, MD, T], f16, kind="ExternalOutput")

    n = len(plan)
    with tile.TileContext(nc) as tc:
        with (
            tc.tile_pool(name="consts", bufs=1) as consts,
            tc.tile_pool(name="w1pool", bufs=2) as w1pool,
            tc.tile_pool(name="dw1pool", bufs=1) as dw1pool,
            tc.tile_pool(name="w2pool", bufs=2) as w2pool,
            tc.tile_pool(name="dw2pool", bufs=1) as dw2pool,
            tc.tile_pool(name="zpool", bufs=2) as zpool,
            tc.tile_pool(name="hpool", bufs=2) as hpool,
            tc.tile_pool(name="hqpool", bufs=2) as hqpool,
            tc.tile_pool(name="opool", bufs=4) as opool,
            tc.tile_pool(name="php", bufs=3, space="PSUM") as php,
            tc.tile_pool(name="pop", bufs=3, space="PSUM") as pop,
        ):
            b1t = consts.tile([128, V, MH], f32)
            nc.sync.dma_start(b1t[:], b1_d[:])

            ztiles = {}
            htiles = {}
            wtiles = {}

            def fetch_z(ti):
                v, toff, nt = plan[ti]
                zt = zpool.tile([128, KC1, 2, NT], f8, tag="zq",
                                name=f"zq_{ti}")
                dzt = zpool.tile([128, KC1, 2, NT], f8, tag="dz",
                                 name=f"dz_{ti}")
                nc.sync.dma_start(zt[:, :, :, :nt],
                                  zq_d[:, :, :, toff:toff + nt])
                nc.sync.dma_start(dzt[:, :, :, :nt],
                                  dz_d[:, :, :, toff:toff + nt])
                ztiles[ti] = (zt, dzt)

            fetch_z(0)
            for ti in range(n + 1):
                if ti < n:
                    v, toff, nt = plan[ti]
                    new_view = v not in wtiles
                    if new_view:
                        w1t = w1pool.tile([128, KC1, 2, HS], f8, tag="w1",
                                          name=f"w1_{v}")
                        dw1t = dw1pool.tile([128, KC1, 2, HS], f8, tag="dw1",
                                            name=f"dw1_{v}")
                        nc.sync.dma_start(w1t[:], w1_d[v])
                        nc.sync.dma_start(dw1t[:], dw1_d[v])
                        wtiles[v] = {"w1": w1t, "dw1": dw1t}
                    wt = wtiles[v]
                    zt, dzt = ztiles.pop(ti)
                    h32 = hpool.tile([128, MH, NT], bf16, tag="h32",
                                     name=f"h32_{ti}")
                    hqt = hqpool.tile([128, KC2, 2, NT], f8, tag="hq",
                                      name=f"hq_{ti}")
                    for m in range(MH):
                        if m == 1 and ti + 1 < n:
                            fetch_z(ti + 1)
                        ph = php.tile([128, NT], f32, tag="ph")
                        nmm = 3 * KC1
                        for kc in range(KC1):
                            i = 3 * kc
                            w1s = wt["w1"][:, kc, :, bass.ts(m, 128)]
                            dw1s = wt["dw1"][:, kc, :, bass.ts(m, 128)]
                            zs = zt[:, kc, :, :nt]
                            dzs = dzt[:, kc, :, :nt]
                            nc.tensor.matmul(ph[:, :nt], w1s, zs,
                                             start=(i == 0),
                                             stop=(i == nmm - 1), perf_mode=DR)
                            nc.tensor.matmul(ph[:, :nt], w1s, dzs,
                                             start=False,
                                             stop=(i + 1 == nmm - 1),
                                             perf_mode=DR)
                            nc.tensor.matmul(ph[:, :nt], dw1s, zs,
                                             start=False,
                                             stop=(i + 2 == nmm - 1),
                                             perf_mode=DR)
                        nc.scalar.activation(h32[:, m, :nt], ph[:, :nt],
                                             mybir.ActivationFunctionType.Gelu,
                                             bias=b1t[:, v, m:m + 1],
                                             scale=1.0 / SW)
                        if m == MH // 2 - 1 or m == MH - 1:
                            lo = 0 if m == MH // 2 - 1 else MH // 2
                            hq_ap = hqt[:, lo // 2:(lo + MH // 2) // 2, :, :nt]
                            nc.vector.tensor_scalar_sub(
                                hq_ap.rearrange("p a b n -> p (a b) n"),
                                h32[:, lo:lo + MH // 2, :nt], C_SHIFT)
                    htiles[ti] = hqt
                    if new_view:
                        w2t = w2pool.tile([128, KC2, 2, D], f8, tag="w2",
                                          name=f"w2_{v}")
                        dw2t = dw2pool.tile([128, KC2, 2, D], f8, tag="dw2",
                                            name=f"dw2_{v}")
                        nc.sync.dma_start(w2t[:], w2_d[v])
                        nc.sync.dma_start(dw2t[:], dw2_d[v])
                        wt["w2"] = w2t
                        wt["dw2"] = dw2t

                if ti >= 1:
                    v1, toff1, nt1 = plan[ti - 1]
                    hqt = htiles.pop(ti - 1)
                    wt = wtiles[v1]
                    for dsub in range(MD):
                        po = pop.tile([128, NT], f32, tag="po")
                        for kc2 in range(KC2):
                            i = 2 * kc2
                            w2s = wt["w2"][:, kc2, :, bass.ts(dsub, 128)]
                            dw2s = wt["dw2"][:, kc2, :, bass.ts(dsub, 128)]
                            hs = hqt[:, kc2, :, :nt1]
                            nc.tensor.matmul(po[:, :nt1], w2s, hs,
                                             start=(i == 0),
                                             stop=(i == 2 * KC2 - 1),
                                             perf_mode=DR)
                            nc.tensor.matmul(po[:, :nt1], dw2s, hs,
                                             start=False,
                                             stop=(i + 1 == 2 * KC2 - 1),
                                             perf_mode=DR)
                        ot = opool.tile([128, NT], f16, tag="ot")
                        nc.vector.tensor_copy(ot[:, :nt1], po[:, :nt1])
                        nc.sync.dma_start(out_d[:, dsub, toff1:toff1 + nt1],
                                          ot[:, :nt1])
    nc.finalize()
    return nc


def _pack_k(a):
    """[K, X] -> [128, K//256, 2, X] DoubleRow pair layout (k = kc*256
    + slot*128 + partition)."""
    K_, X = a.shape
    return np.ascontiguousarray(
        a.reshape(K_ // 256, 2, 128, X).transpose(2, 0, 1, 3))


def kernel(**inputs):
    x = np.asarray(inputs["vision_features"], dtype=np.float32)    # [B, P, D]
    idx = np.asarray(inputs["student_view_indices"]).astype(np.int64)  # [B]
    gamma = np.asarray(inputs["gamma"], dtype=np.float32)          # [V, D]
    beta = np.asarray(inputs["beta"], dtype=np.float32)            # [V, D]
    W1 = np.asarray(inputs["W1"], dtype=np.float32)                # [V, D, H]
    b1 = np.asarray(inputs["b1"], dtype=np.float32)                # [V, H]
    W2 = np.asarray(inputs["W2"], dtype=np.float32)                # [V, H, D]
    b2 = np.asarray(inputs["b2"], dtype=np.float32)                # [V, D]

    order = np.argsort(idx, kind="stable")
    idx_sorted = idx[order]
    plan = _tile_plan(idx_sorted)

    # host-side folds: gamma into W1 rows, beta into b1
    W1f = gamma[:, :, None] * W1                                   # [V, D, H]
    b1f = b1 + np.einsum("vd,vdh->vh", beta, W1)                   # [V, H]

    xs = x[order].reshape(T, D)                                    # sorted tokens

    # per-token LayerNorm (fp64 stats), then fp8 pair quantization of z
    mu_t = xs.mean(axis=1, dtype=np.float64)
    ex2 = np.einsum("td,td->t", xs.astype(np.float64), xs.astype(np.float64)) / D
    var = ex2 - mu_t * mu_t
    rstd_t = (1.0 / np.sqrt(var + LN_EPS)).astype(np.float32)
    z = (xs - mu_t.astype(np.float32)[:, None]) * rstd_t[:, None]  # [T, D]
    zq = z.astype(E4)
    dz = (z - zq.astype(np.float32)).astype(E4)
    zq_p = _pack_k(zq.astype(np.float32).T).astype(E4)             # [128,KC1,2,T]
    dz_p = _pack_k(dz.astype(np.float32).T).astype(E4)

    # weight quantization (x64 prescale) + DoubleRow packing
    w1q = (W1f * SW).astype(E4)
    dw1 = (W1f * SW - w1q.astype(np.float32)).astype(E4)
    w2q = (W2 * SW).astype(E4)
    dw2 = (W2 * SW - w2q.astype(np.float32)).astype(E4)
    # host-exact pieces of the reconstruction
    w2eff_colsum = (w2q.astype(np.float32) + dw2.astype(np.float32)).sum(1) / SW

    w1q_p = np.stack([_pack_k(w1q[v].astype(np.float32)) for v in range(V)])
    dw1_p = np.stack([_pack_k(dw1[v].astype(np.float32)) for v in range(V)])
    w2q_p = np.stack([_pack_k(w2q[v].astype(np.float32)) for v in range(V)])
    dw2_p = np.stack([_pack_k(dw2[v].astype(np.float32)) for v in range(V)])
    # -> [V, 128, D//256|H//256, 2, H|D]; slice the HS range per core
    b1_p = b1f.reshape(V, NCORES, MH, 128)

    in_maps = []
    for c in range(NCORES):
        hsl = slice(c * HS, (c + 1) * HS)
        csl = slice(c * KC2, (c + 1) * KC2)
        in_maps.append({
            "zq": zq_p,
            "dz": dz_p,
            "w1q": np.ascontiguousarray(w1q_p[:, :, :, :, hsl]).astype(E4),
            "dw1": np.ascontiguousarray(dw1_p[:, :, :, :, hsl]).astype(E4),
            "w2q": np.ascontiguousarray(w2q_p[:, :, csl]).astype(E4),
            "dw2": np.ascontiguousarray(dw2_p[:, :, csl]).astype(E4),
            "b1": np.ascontiguousarray(b1_p[:, c].transpose(2, 0, 1)),
        })

    nc = build(plan)
    res = run_bass_kernel_spmd(nc, in_maps, core_ids=list(range(NCORES)))
    global LAST_NC, LAST_RESULT
    LAST_NC = nc
    LAST_RESULT = res

    acc = res.results[0]["pout"].astype(np.float32).copy()
    for c in range(1, NCORES):
        acc += res.results[c]["pout"].astype(np.float32)
    pout = acc.transpose(1, 0, 2).reshape(D, T)                    # [D, T]

    tok_view = np.repeat(idx_sorted, P)
    out_sorted = xs + pout.T / SW
    out_sorted += C_SHIFT * w2eff_colsum[tok_view] + b2[tok_view]
    out = np.empty((B, P, D), dtype=np.float32)
    out[order] = out_sorted.reshape(B, P, D)
    return out


# revision 19
# speedup vs baseline: 1.5976x; 1.0807x over previous
"""Trainium2 Bass kernel for nn_DistillationStudentModel (per-view adapter MLP).

Math (per sample b with view v = idx[b]):
    xn  = LayerNorm(x; gamma[v], beta[v])
    h   = gelu(xn @ W1[v] + b1[v])          (erf gelu)
    out = x + h @ W2[v] + b2[v]

Strategy: shard the MLP hidden dim H=8192 across the 8 cores (HS=1024 each);
every core processes all tokens against its H-slice and emits a partial mm2
output that the host reduces (plus residual x and biases).

The matmuls run in fp8-e4m3 with perf_mode=DoubleRow (K=256 per matmul, 2
PE MACs/cell/cycle).  Plain fp8 is far outside the 2e-2 tolerance, so each
matmul is augmented with first-order correction matmuls:

    z @ W  ~=  Q(z)@Q(W) + Q(z - Q(z))@Q(W) + Q(z)@Q(W*64 - Q(W*64))/64

mm1 keeps both corrections (activation delta dz and weight delta dW1) ->
numerically exact to ~1e-3.  mm2 keeps only the weight correction dW2; the
h-quantization error is shrunk by quantizing (h - 0.3) (the gelu output is
skewed positive, so a constant shift reduces relative quantization error)
and the dropped shift term 0.3*colsum(W2) is added back exactly on the host.
Measured end-to-end rel err ~1.7e-2 (budget 2e-2) vs 874us -> ~550us of PE
time (mm1: 1.5 cyc per K=256 column, mm2: 1.0 vs bf16's 2.0).

Weights are pre-scaled by 64 (power of two, exact) so fp8's normal range
covers them; the 1/64 is folded into the gelu's input scale and the host
reduction.  LayerNorm stats and quantization run on the host; z arrives
pre-packed as fp8 pairs.  Samples are sorted by view so each view's weights
are loaded once; the token-tile plan is baked into the compiled program.
The per-tile schedule is software-pipelined one tile deep (mm1 of tile i+1
is emitted before mm2 of tile i) so the PE never waits for the
gelu->quantize chain.
"""

import numpy as np
import ml_dtypes

import concourse.bass as bass
import concourse.tile as tile
from concourse import bacc, mybir
from concourse.bass_utils import run_bass_kernel_spmd

B, P, D, H, V = 32, 256, 2048, 8192, 3
NCORES = 8
HS = H // NCORES          # per-core hidden slice
T = B * P                 # total tokens
KC1 = D // 256            # mm1 DoubleRow contraction chunks
MH = HS // 128            # mm1 output row tiles
KC2 = HS // 256           # mm2 DoubleRow contraction chunks
MD = D // 128             # mm2 output row tiles
NT = 512                  # tokens per tile (2 samples)
LN_EPS = 1e-5
SW = 64.0                 # weight prescale (power of two)
C_SHIFT = 0.30            # h quantization shift

f32 = mybir.dt.float32
bf16 = mybir.dt.bfloat16
f16 = mybir.dt.float16
f8 = mybir.dt.float8e4
E4 = ml_dtypes.float8_e4m3
DR = mybir.MatmulPerfMode.DoubleRow

# debugging/profiling hooks (unused by the grading path)
LAST_NC = None
LAST_RESULT = None


def _tile_plan(idx_sorted):
    """[(view, tok_offset, n_tokens)] with n_tokens in {512, 256}, aligned to
    sorted sample groups so every tile is single-view."""
    counts = np.bincount(idx_sorted, minlength=V)
    plan = []
    off = 0
    for v in range(V):
        n = int(counts[v])
        for _ in range(n // 2):
            plan.append((v, off, 2 * P))
            off += 2 * P
        if n % 2:
            plan.append((v, off, P))
            off += P
    assert off == T
    return plan


def build(plan):
    nc = bacc.Bacc("TRN2", debug=False, num_devices=NCORES)
    zq_d = nc.dram_tensor("zq", [128, KC1, 2, T], f8, kind="ExternalInput")
    dz_d = nc.dram_tensor("dz", [128, KC1, 2, T], f8, kind="ExternalInput")
    w1_d = nc.dram_tensor("w1q", [V, 128, MH, KC1, 2, 128], f8,
                          kind="ExternalInput")
    dw1_d = nc.dram_tensor("dw1", [V, 128, MH, KC1, 2, 128], f8,
                           kind="ExternalInput")
    w2_d = nc.dram_tensor("w2q", [V, 128, KC2, 2, D], f8, kind="ExternalInput")
    dw2_d = nc.dram_tensor("dw2", [V, 128, KC2 - 1, 2, D], f8,
                           kind="ExternalInput")
    b1_d = nc.dram_tensor("b1", [128, V, MH], f32, kind="ExternalInput")
    out_d = nc.dram_tensor("pout", [128, MD, T], f16, kind="ExternalOutput")

    n = len(plan)
    with tile.TileContext(nc) as tc:
        with (
            tc.tile_pool(name="consts", bufs=1) as consts,
            tc.tile_pool(name="w1pool", bufs=2) as w1pool,
            tc.tile_pool(name="dw1pool", bufs=1) as dw1pool,
            tc.tile_pool(name="w2pool", bufs=2) as w2pool,
            tc.tile_pool(name="dw2pool", bufs=1) as dw2pool,
            tc.tile_pool(name="zpool", bufs=2) as zpool,
            tc.tile_pool(name="hpool", bufs=2) as hpool,
            tc.tile_pool(name="hqpool", bufs=2) as hqpool,
            tc.tile_pool(name="opool", bufs=4) as opool,
            tc.tile_pool(name="php", bufs=4, space="PSUM") as php,
            tc.tile_pool(name="pop", bufs=4, space="PSUM") as pop,
        ):
            b1t = consts.tile([128, V, MH], f32)
            nc.sync.dma_start(b1t[:], b1_d[:])

            ztiles = {}
            htiles = {}
            wtiles = {}

            def fetch_z(ti):
                v, toff, nt = plan[ti]
                zt = zpool.tile([128, KC1, 2, NT], f8, tag="zq",
                                name=f"zq_{ti}")
                dzt = zpool.tile([128, KC1, 2, NT], f8, tag="dz",
                                 name=f"dz_{ti}")
                nc.sync.dma_start(zt[:, :, :, :nt],
                                  zq_d[:, :, :, toff:toff + nt])
                nc.sync.dma_start(dzt[:, :, :, :nt],
                                  dz_d[:, :, :, toff:toff + nt])
                ztiles[ti] = (zt, dzt)

            fetch_z(0)
            for ti in range(n + 1):
                if ti < n:
                    v, toff, nt = plan[ti]
                    new_view = v not in wtiles
                    if new_view:
                        # per-m DMAs so the first m-tile's matmuls start as
                        # soon as 1/8th of the weights have landed
                        w1t = w1pool.tile([128, MH, KC1, 2, 128], f8, tag="w1",
                                          name=f"w1_{v}")
                        dw1t = dw1pool.tile([128, MH, KC1, 2, 128], f8,
                                            tag="dw1", name=f"dw1_{v}")
                        for m in range(MH):
                            nc.sync.dma_start(w1t[:, m], w1_d[v, :, m])
                            nc.sync.dma_start(dw1t[:, m], dw1_d[v, :, m])
                        wtiles[v] = {"w1": w1t, "dw1": dw1t}
                    wt = wtiles[v]
                    zt, dzt = ztiles.pop(ti)
                    h32 = hpool.tile([128, MH, NT], bf16, tag="h32",
                                     name=f"h32_{ti}")
                    hqt = hqpool.tile([128, KC2, 2, NT], f8, tag="hq",
                                      name=f"hq_{ti}")
                    for m in range(MH):
                        if m == 1 and ti + 1 < n:
                            fetch_z(ti + 1)
                        ph = php.tile([128, NT], f32, tag="ph")
                        nmm = 3 * KC1
                        for kc in range(KC1):
                            i = 3 * kc
                            w1s = wt["w1"][:, m, kc, :, :]
                            dw1s = wt["dw1"][:, m, kc, :, :]
                            zs = zt[:, kc, :, :nt]
                            dzs = dzt[:, kc, :, :nt]
                            nc.tensor.matmul(ph[:, :nt], w1s, zs,
                                             start=(i == 0),
                                             stop=(i == nmm - 1), perf_mode=DR)
                            nc.tensor.matmul(ph[:, :nt], w1s, dzs,
                                             start=False,
                                             stop=(i + 1 == nmm - 1),
                                             perf_mode=DR)
                            nc.tensor.matmul(ph[:, :nt], dw1s, zs,
                                             start=False,
                                             stop=(i + 2 == nmm - 1),
                                             perf_mode=DR)
                        nc.scalar.activation(h32[:, m, :nt], ph[:, :nt],
                                             mybir.ActivationFunctionType.Gelu,
                                             bias=b1t[:, v, m:m + 1],
                                             scale=1.0 / SW)
                        if m == MH // 2 - 1 or m == MH - 1:
                            lo = 0 if m == MH // 2 - 1 else MH // 2
                            hq_ap = hqt[:, lo // 2:(lo + MH // 2) // 2, :, :nt]
                            nc.scalar.activation(
                                hq_ap.rearrange("p a b n -> p (a b) n"),
                                h32[:, lo:lo + MH // 2, :nt],
                                mybir.ActivationFunctionType.Copy,
                                bias=-C_SHIFT, scale=1.0)
                    htiles[ti] = hqt
                    if new_view:
                        w2t = w2pool.tile([128, KC2, 2, D], f8, tag="w2",
                                          name=f"w2_{v}")
                        dw2t = dw2pool.tile([128, KC2 - 1, 2, D], f8,
                                            tag="dw2", name=f"dw2_{v}")
                        nc.sync.dma_start(w2t[:], w2_d[v])
                        nc.sync.dma_start(dw2t[:], dw2_d[v])
                        wt["w2"] = w2t
                        wt["dw2"] = dw2t

                if ti >= 1:
                    v1, toff1, nt1 = plan[ti - 1]
                    hqt = htiles.pop(ti - 1)
                    wt = wtiles[v1]
                    for dsub in range(MD):
                        po = pop.tile([128, NT], f32, tag="po")
                        # kc2=0 skips the dw2 correction (error-budget spend)
                        nmm2 = 2 * KC2 - 1
                        i = 0
                        for kc2 in range(KC2):
                            w2s = wt["w2"][:, kc2, :, bass.ts(dsub, 128)]
                            hs = hqt[:, kc2, :, :nt1]
                            nc.tensor.matmul(po[:, :nt1], w2s, hs,
                                             start=(i == 0),
                                             stop=(i == nmm2 - 1),
                                             perf_mode=DR)
                            i += 1
                            if kc2 == 0:
                                continue
                            dw2s = wt["dw2"][:, kc2 - 1, :, bass.ts(dsub, 128)]
                            nc.tensor.matmul(po[:, :nt1], dw2s, hs,
                                             start=False,
                                             stop=(i == nmm2 - 1),
                                             perf_mode=DR)
                            i += 1
                        ot = opool.tile([128, NT], f16, tag="ot")
                        if dsub % 2:
                            nc.scalar.activation(
                                ot[:, :nt1], po[:, :nt1],
                                mybir.ActivationFunctionType.Copy,
                                bias=0.0, scale=1.0)
                        else:
                            nc.vector.tensor_copy(ot[:, :nt1], po[:, :nt1])
                        nc.sync.dma_start(out_d[:, dsub, toff1:toff1 + nt1],
                                          ot[:, :nt1])
    nc.finalize()
    return nc


def _pack_k(a):
    """[K, X] -> [128, K//256, 2, X] DoubleRow pair layout (k = kc*256
    + slot*128 + partition)."""
    K_, X = a.shape
    return np.ascontiguousarray(
        a.reshape(K_ // 256, 2, 128, X).transpose(2, 0, 1, 3))


def kernel(**inputs):
    x = np.asarray(inputs["vision_features"], dtype=np.float32)    # [B, P, D]
    idx = np.asarray(inputs["student_view_indices"]).astype(np.int64)  # [B]
    gamma = np.asarray(inputs["gamma"], dtype=np.float32)          # [V, D]
    beta = np.asarray(inputs["beta"], dtype=np.float32)            # [V, D]
    W1 = np.asarray(inputs["W1"], dtype=np.float32)                # [V, D, H]
    b1 = np.asarray(inputs["b1"], dtype=np.float32)                # [V, H]
    W2 = np.asarray(inputs["W2"], dtype=np.float32)                # [V, H, D]
    b2 = np.asarray(inputs["b2"], dtype=np.float32)                # [V, D]

    order = np.argsort(idx, kind="stable")
    idx_sorted = idx[order]
    plan = _tile_plan(idx_sorted)

    # host-side folds: gamma into W1 rows, beta into b1
    W1f = gamma[:, :, None] * W1                                   # [V, D, H]
    b1f = b1 + np.einsum("vd,vdh->vh", beta, W1)                   # [V, H]

    xs = x[order].reshape(T, D)                                    # sorted tokens

    # per-token LayerNorm (fp64 stats), then fp8 pair quantization of z
    mu_t = xs.mean(axis=1, dtype=np.float64)
    ex2 = np.einsum("td,td->t", xs.astype(np.float64), xs.astype(np.float64)) / D
    var = ex2 - mu_t * mu_t
    rstd_t = (1.0 / np.sqrt(var + LN_EPS)).astype(np.float32)
    z = (xs - mu_t.astype(np.float32)[:, None]) * rstd_t[:, None]  # [T, D]
    zq = z.astype(E4)
    dz = (z - zq.astype(np.float32)).astype(E4)
    zq_p = _pack_k(zq.astype(np.float32).T).astype(E4)             # [128,KC1,2,T]
    dz_p = _pack_k(dz.astype(np.float32).T).astype(E4)

    # weight quantization (x64 prescale) + DoubleRow packing
    w1q = (W1f * SW).astype(E4)
    dw1 = (W1f * SW - w1q.astype(np.float32)).astype(E4)
    w2q = (W2 * SW).astype(E4)
    dw2 = (W2 * SW - w2q.astype(np.float32)).astype(E4)
    # host-exact pieces of the reconstruction: effective W2 drops the dw2
    # correction on each core's first 256 hidden rows (kc2=0)
    w2eff = w2q.astype(np.float32) + dw2.astype(np.float32)
    kc0 = (np.arange(H) % HS) < 256
    w2eff[:, kc0, :] -= dw2.astype(np.float32)[:, kc0, :]
    w2eff_colsum = w2eff.sum(1) / SW
    del w2eff

    w1q_p = np.stack([_pack_k(w1q[v].astype(np.float32)) for v in range(V)])
    dw1_p = np.stack([_pack_k(dw1[v].astype(np.float32)) for v in range(V)])
    # -> [V, 128, KC1, 2, H]; reorganize the H axis m-major per core below
    w2q_p = np.stack([_pack_k(w2q[v].astype(np.float32)) for v in range(V)])
    dw2_p = np.stack([_pack_k(dw2[v].astype(np.float32)) for v in range(V)])
    # -> [V, 128, D//256|H//256, 2, H|D]; slice the HS range per core
    b1_p = b1f.reshape(V, NCORES, MH, 128)

    def _mmajor(a, hsl):
        # [V,128,KC1,2,HS-slice] -> [V,128,MH,KC1,2,128]
        s = a[:, :, :, :, hsl]
        return np.ascontiguousarray(
            s.reshape(V, 128, KC1, 2, MH, 128).transpose(0, 1, 4, 2, 3, 5))

    in_maps = []
    for c in range(NCORES):
        hsl = slice(c * HS, (c + 1) * HS)
        csl = slice(c * KC2, (c + 1) * KC2)
        in_maps.append({
            "zq": zq_p,
            "dz": dz_p,
            "w1q": _mmajor(w1q_p, hsl).astype(E4),
            "dw1": _mmajor(dw1_p, hsl).astype(E4),
            "w2q": np.ascontiguousarray(w2q_p[:, :, csl]).astype(E4),
            "dw2": np.ascontiguousarray(
                dw2_p[:, :, c * KC2 + 1:(c + 1) * KC2]).astype(E4),
            "b1": np.ascontiguousarray(b1_p[:, c].transpose(2, 0, 1)),
        })

    nc = build(plan)
    res = run_bass_kernel_spmd(nc, in_maps, core_ids=list(range(NCORES)))
    global LAST_NC, LAST_RESULT
    LAST_NC = nc
    LAST_RESULT = res

    acc = res.results[0]["pout"].astype(np.float32).copy()
    for c in range(1, NCORES):
        acc += res.results[c]["pout"].astype(np.float32)
    pout = acc.transpose(1, 0, 2).reshape(D, T)                    # [D, T]

    tok_view = np.repeat(idx_sorted, P)
    out_sorted = xs + pout.T / SW
    out_sorted += C_SHIFT * w2eff_colsum[tok_view] + b2[tok_view]
    out = np.empty((B, P, D), dtype=np.float32)
    out[order] = out_sorted.reshape(B, P, D)
    return out
